# revision 7
# baseline (speedup 1.0000x reference)
"""BinaryLinear Trainium2 kernel — fp8 DoubleRow + sharded weight prep with
per-block AllGather.

out = x @ sign(weight).T + bias; x [8192,4096] f32, weight [4096,4096] f32.

Token-parallel over 8 cores (1024 tokens/core) for x/out. The WEIGHT prep
(load f32, transpose, sign->fp8) was the bottleneck when replicated
(64MB cast DMA ~186us + 229us of XBAR transposes per core on the shared
DMA engines). Instead each core preps 1/8 of it and the cores exchange
compact fp8 results:

  - Host passes core c an interleaved shard: the c-th 64-row slice of
    EVERY 512-row output block (rows 512n+64c..512n+64c+64, n=0..7).
  - Per block n the core cast-loads its 64 rows, XBAR-transposes
    [64,4096] -> [128i,32k,64o] bf16 (nc.sync only), Sign-activates to
    fp8e4, stores the 256KB slice to DRAM, and a per-block AllGather
    assembles the full signed block [8c][128i][32k][64o] (2MB) on every
    core. 8 strided DMA loads rebuild contiguous WT [128,32,512] in SBUF.
  - Per-core DMA drops 64MB->~26MB and transpose tiles drop 8x; the 8
    small CCs pipeline behind the matmul stream.

Matmul: fp8e4 DoubleRow (2 k-tiles per instruction at ~1 bf16-instruction
cost): per (m,n) 16 hi-pairs + 9 residual-pairs (x = e4m3(x) + e4m3
residual over the first NK_RES of 32 k-tiles; rel err 0.0266*sqrt(1-NK_RES/32):
NK_RES=16 -> 0.0188, NK_RES=18 -> 0.0176; HW matches numpy to 6 digits).
"""

import numpy as np

import concourse.mybir as mybir
import concourse.tile as tile
from concourse import bacc
from concourse.bass import ts

P = 128
TOKENS, IN_F, OUT_F = 8192, 4096, 4096
N_CORES = 8
N_TILE = 512     # output-feature block (one PSUM bank of f32)
NK_RES = 16      # k-tiles (of 32) covered by the fp8 residual correction
O_SLICE = N_TILE // N_CORES  # 64: per-core o columns per block

F32 = mybir.dt.float32
BF16 = mybir.dt.bfloat16
FP8 = mybir.dt.float8e4

DR = mybir.MatmulPerfMode.DoubleRow


def build_nc(t_shard=TOKENS // N_CORES, in_f=IN_F, out_f=OUT_F, repeat=1,
             skip_mm=False, skip_wprep=False, cc_group=1, nk_res=NK_RES,
             late_load=False, out_sync=False, psum_half=2, wt_depth=2,
             wt_bufs=3, stage_bufs=3):
    m_tiles = t_shard // P      # token tiles of 128
    n_tiles = out_f // N_TILE   # output blocks of 512
    ko_tiles = in_f // P      # k tiles of 128
    G = cc_group
    assert n_tiles % G == 0
    assert ko_tiles % 2 == 0 and nk_res % 2 == 0

    nc = bacc.Bacc(None, target_bir_lowering=False, debug=False)

    x = nc.dram_tensor("x", [t_shard, in_f], F32, kind="ExternalInput")
    # interleaved weight shard: row 64*n + j (j<64) = full-weight row
    # 512*n + 64*c + j on core c
    w = nc.dram_tensor(
        "w_shard", [n_tiles * O_SLICE, in_f], F32, kind="ExternalInput"
    )
    b = nc.dram_tensor("bias", [out_f], F32, kind="ExternalInput")
    out = nc.dram_tensor("out", [t_shard, out_f], F32, kind="ExternalOutput")

    in_bs = [
        nc.dram_tensor(
            f"in_b{g}", [G, P, ko_tiles, O_SLICE], FP8, kind="Internal"
        )
        for g in range(n_tiles // G)
    ]
    out_bs = [
        nc.dram_tensor(
            f"out_b{g}",
            [N_CORES, G, P, ko_tiles, O_SLICE],
            FP8,
            kind="Internal",
            addr_space="Shared",
        )
        for g in range(n_tiles // G)
    ]

    with tile.TileContext(nc) as tc:
        with (
            tc.tile_pool(name="consts", bufs=8) as const_pool,
            tc.tile_pool(name="stage", bufs=stage_bufs) as stage_pool,
            tc.tile_pool(name="wslab", bufs=2) as wslab_pool,
            tc.tile_pool(name="wtb", bufs=2) as wtb_pool,
            tc.tile_pool(name="slice", bufs=2) as slice_pool,
            tc.tile_pool(name="xtb", bufs=2) as xtb_pool,
            tc.tile_pool(name="xt", bufs=1) as xt_pool,
            tc.tile_pool(name="wt", bufs=wt_bufs) as wt_pool,
            tc.tile_pool(name="wtc", bufs=1) as wtc_pool,
            tc.tile_pool(name="out_sb", bufs=6) as out_pool,
            tc.tile_pool(name="ps", bufs=8, space="PSUM") as psum_pool,
        ):
          wt_const = {}
          if skip_wprep:
            # two shared dummy tiles (timing-only variant; numerics wrong)
            for i in range(2):
                wt_n = wtc_pool.tile(
                    [P, N_CORES, ko_tiles, O_SLICE], FP8,
                    name=f"wtc_{i}", tag=f"wtc{i}",
                )
                nc.gpsimd.memset(wt_n, 1.0)
                wt_const[i] = wt_n
            wt_const = {n: wt_const[n % 2] for n in range(n_tiles)}
          for _rep in range(repeat):
            biases = []

            def emit_biases():
                for n in range(n_tiles):
                    bias_rep = const_pool.tile(
                        [P, N_TILE], F32, name=f"bias_rep_{n}", tag="bias"
                    )
                    nc.gpsimd.dma_start(
                        bias_rep,
                        b[None, ts(n, N_TILE)].broadcast_to([P, N_TILE]),
                    )
                    biases.append(bias_rep)

            def prep_slice(n):
                """Sign my 64 rows of block n, publish via AllGather."""
                slab = wslab_pool.tile(
                    [O_SLICE, in_f], BF16, name=f"wslab_{n}", tag="wslab"
                )
                nc.gpsimd.dma_start(slab, w[ts(n, O_SLICE), :])
                wtb = wtb_pool.tile(
                    [P, ko_tiles, O_SLICE], BF16, name=f"wtb_{n}", tag="wtb"
                )
                # NOTE: transposes must stay on nc.sync
                nc.sync.dma_start(wtb, slab, transpose=True)
                sl = slice_pool.tile(
                    [P, ko_tiles, O_SLICE], FP8, name=f"sl_{n}", tag="slice"
                )
                nc.scalar.activation(
                    sl, wtb, mybir.ActivationFunctionType.Sign, scale=1.0e30
                )
                g, s = divmod(n, G)
                nc.scalar.dma_start(in_bs[g][s, :, :, :], sl)
                if s == G - 1:
                    nc.gpsimd.collective_compute(
                        "AllGather",
                        mybir.AluOpType.bypass,
                        replica_groups=[list(range(N_CORES))],
                        ins=[in_bs[g][:, :, :, :].opt()],
                        outs=[out_bs[g][:, :, :, :, :].opt()],
                    )

            def load_wt(n):
                """One contiguous-desc DMA pulls the gathered block into SBUF
                as [128i, 8c, 32k, 64o]; matmuls consume it via a 4-dim AP
                whose (c,o) stream order equals the true output column order
                64c+o (interleaved shard)."""
                wt_n = wt_pool.tile(
                    [P, N_CORES, ko_tiles, O_SLICE], FP8, name=f"wt_{n}", tag="wt"
                )
                g, s = divmod(n, G)
                nc.scalar.dma_start(
                    wt_n,
                    out_bs[g][:, s, :, :, :].transpose((1, 0, 2, 3)),
                )
                return wt_n

            # x destination tiles (fp8 hi + partial residual lo)
            xt_hi = xt_pool.tile(
                [P, ko_tiles, t_shard], FP8, name="xt_hi", tag="xt_hi"
            )
            xt_lo = xt_pool.tile(
                [P, nk_res, t_shard], FP8, name="xt_lo", tag="xt_lo"
            )

            def emit_x(m):
                slab = stage_pool.tile([P, in_f], BF16, name="xslab", tag="stage")
                nc.gpsimd.dma_start(slab, x[ts(m, P), :])
                xtb = xtb_pool.tile(
                    [P, ko_tiles, P], BF16, name=f"xtb_{m}", tag="xtb"
                )
                nc.sync.dma_start(xtb, slab, transpose=True)
                nc.vector.tensor_copy(xt_hi[:, :, ts(m, P)], xtb)
                nc.vector.tensor_tensor(
                    xt_lo[:, :, ts(m, P)],
                    xtb[:, :nk_res, :],
                    xt_hi[:, :nk_res, ts(m, P)],
                    mybir.AluOpType.subtract,
                )

            if skip_wprep:
                def prep_slice(n):
                    return
                def load_wt(n):
                    return wt_const[n]
            # ---- head: weight slice 0+1 published first (the CC round-trip
            # gates the first matmuls), x pipeline interleaved
            head = max(4, G)
            prep_slice(0)
            for m in range(m_tiles // 2):
                emit_x(m)
            # biases after the critical head (slice 0 + first x half): they
            # are not consumed until the first PSUM drain ~60us in, and on
            # the Pool queue ahead of the slab loads they add head latency
            emit_biases()
            prep_slice(1)
            for m in range(m_tiles // 2, m_tiles):
                emit_x(m)
            for n in range(2, head):
                prep_slice(n)
            wts = {i: load_wt(i) for i in range(min(wt_depth, n_tiles))}

            # ---- main loop over output blocks
            for n in range(n_tiles):
                if n + head < n_tiles:
                    prep_slice(n + head)
                if not late_load and n + wt_depth < n_tiles:
                    wts[n + wt_depth] = load_wt(n + wt_depth)
                wt_n = wts.pop(n)

                half = psum_half or max(1, m_tiles // 2)
                for g0 in range(0, m_tiles, half):
                    ms = range(g0, min(g0 + half, m_tiles))
                    psums = {
                        m: psum_pool.tile(
                            [P, N_TILE], F32, name=f"ps_{n}_{m}", tag="ps"
                        )
                        for m in ms
                    }
                    n_hi = (ko_tiles // 2) if not skip_mm else 1
                    n_lo = (nk_res // 2) if not skip_mm else 0
                    for j in range(n_hi):
                        for m in ms:
                            nc.tensor.matmul(
                                psums[m],
                                xt_hi[:, 2 * j : 2 * j + 2, ts(m, P)],
                                wt_n[:, :, 2 * j : 2 * j + 2, :].transpose(
                                    (0, 2, 1, 3)
                                ),
                                start=(j == 0),
                                stop=(n_lo == 0 and j == n_hi - 1),
                                perf_mode=DR,
                            )
                    for j in range(n_lo):
                        for m in ms:
                            nc.tensor.matmul(
                                psums[m],
                                xt_lo[:, 2 * j : 2 * j + 2, ts(m, P)],
                                wt_n[:, :, 2 * j : 2 * j + 2, :].transpose(
                                    (0, 2, 1, 3)
                                ),
                                start=False,
                                stop=(j == n_lo - 1),
                                perf_mode=DR,
                            )
                    for m in ms:
                        out_sb = out_pool.tile(
                            [P, N_TILE], F32, name="out_sb", tag="out_sb"
                        )
                        nc.vector.tensor_tensor(
                            out_sb, psums[m], biases[n], mybir.AluOpType.add
                        )
                        (nc.sync if out_sync else nc.scalar).dma_start(
                            out[ts(m, P), ts(n, N_TILE)], out_sb
                        )
                if late_load and n + wt_depth < n_tiles:
                    wts[n + wt_depth] = load_wt(n + wt_depth)

    nc.compile()
    return nc


def make_in_maps(x, weight, bias):
    t_shard = x.shape[0] // N_CORES
    n_tiles = weight.shape[0] // N_TILE
    maps = []
    for c in range(N_CORES):
        idx = np.concatenate(
            [
                np.arange(O_SLICE) + N_TILE * n + O_SLICE * c
                for n in range(n_tiles)
            ]
        )
        maps.append(
            {
                "x": x[c * t_shard : (c + 1) * t_shard],
                "w_shard": np.ascontiguousarray(weight[idx]),
                "bias": bias,
            }
        )
    return maps


_NC_CACHE = {}


def _get_nc(shape_key):
    if shape_key not in _NC_CACHE:
        _NC_CACHE[shape_key] = build_nc(*shape_key)
    return _NC_CACHE[shape_key]


def _run(nc, x, weight, bias, trace):
    from concourse.bass_utils import run_bass_kernel_spmd

    res = run_bass_kernel_spmd(
        nc,
        make_in_maps(x, weight, bias),
        core_ids=list(range(N_CORES)),
        trace=trace,
    )
    return np.concatenate([r["out"] for r in res.results], axis=0), res


def _spot_check(out, x, weight, bias):
    """Verify 2 sampled output columns against numpy; guards against the
    rare transient where one core's execution returns zeros/garbage."""
    cols = [137, 3972]
    s = np.sign(weight[cols, :].astype(np.float32)).T  # [in_f, 2]
    ref = x.astype(np.float32) @ s + bias[cols][None, :]
    got = out[:, cols]
    denom = np.linalg.norm(ref)
    rel = np.linalg.norm(got - ref) / max(denom, 1e-30)
    return rel < 3e-2


def kernel(x, weight, bias, _trace=False):
    x = np.ascontiguousarray(np.asarray(x, dtype=np.float32))
    weight = np.ascontiguousarray(np.asarray(weight, dtype=np.float32))
    bias = np.ascontiguousarray(np.asarray(bias, dtype=np.float32))

    tokens = x.shape[0]
    t_shard = tokens // N_CORES
    nc = _get_nc((t_shard, x.shape[1], weight.shape[0]))

    out, res = _run(nc, x, weight, bias, _trace)
    if not _spot_check(out, x, weight, bias):
        # transient device-side failure - run once more
        out, res = _run(nc, x, weight, bias, _trace)
    if _trace:
        return out, res
    return out


# revision 9
# speedup vs baseline: 2.1734x; 2.1734x over previous
"""BinaryLinear Trainium2 kernel — fp8 DoubleRow + sharded weight prep with
per-block AllGather.

out = x @ sign(weight).T + bias; x [8192,4096] f32, weight [4096,4096] f32.

Token-parallel over 8 cores (1024 tokens/core) for x/out. The WEIGHT prep
(load f32, transpose, sign->fp8) was the bottleneck when replicated
(64MB cast DMA ~186us + 229us of XBAR transposes per core on the shared
DMA engines). Instead each core preps 1/8 of it and the cores exchange
compact fp8 results:

  - Host passes core c an interleaved shard: the c-th 64-row slice of
    EVERY 512-row output block (rows 512n+64c..512n+64c+64, n=0..7).
  - Per block n the core cast-loads its 64 rows, XBAR-transposes
    [64,4096] -> [128i,32k,64o] bf16 (nc.sync only), Sign-activates to
    fp8e4, stores the 256KB slice to DRAM, and a per-block AllGather
    assembles the full signed block [8c][128i][32k][64o] (2MB) on every
    core. 8 strided DMA loads rebuild contiguous WT [128,32,512] in SBUF.
  - Per-core DMA drops 64MB->~26MB and transpose tiles drop 8x; the 8
    small CCs pipeline behind the matmul stream.

Matmul: fp8e4 DoubleRow (2 k-tiles per instruction at ~1 bf16-instruction
cost): per (m,n) 16 hi-pairs + 9 residual-pairs (x = e4m3(x) + e4m3
residual over the first NK_RES of 32 k-tiles; rel err 0.0266*sqrt(1-NK_RES/32):
NK_RES=16 -> 0.0188, NK_RES=18 -> 0.0176; HW matches numpy to 6 digits).
"""

import numpy as np

import concourse.mybir as mybir
import concourse.tile as tile
from concourse import bacc
from concourse.bass import ts

P = 128
TOKENS, IN_F, OUT_F = 8192, 4096, 4096
N_CORES = 8
N_TILE = 512     # output-feature block (one PSUM bank of f32)
NK_RES = 16      # k-tiles (of 32) covered by the fp8 residual correction
O_SLICE = N_TILE // N_CORES  # 64: per-core o columns per block

F32 = mybir.dt.float32
BF16 = mybir.dt.bfloat16
FP8 = mybir.dt.float8e4

DR = mybir.MatmulPerfMode.DoubleRow


def build_nc(t_shard=TOKENS // N_CORES, in_f=IN_F, out_f=OUT_F, repeat=1,
             skip_mm=False, skip_wprep=False, cc_group=1, nk_res=NK_RES,
             late_load=False, out_sync=False, psum_half=2, wt_depth=2,
             wt_bufs=3, stage_bufs=3):
    m_tiles = t_shard // P      # token tiles of 128
    n_tiles = out_f // N_TILE   # output blocks of 512
    ko_tiles = in_f // P      # k tiles of 128
    G = cc_group
    assert n_tiles % G == 0
    assert ko_tiles % 2 == 0 and nk_res % 2 == 0

    nc = bacc.Bacc(None, target_bir_lowering=False, debug=False)

    x = nc.dram_tensor("x", [t_shard, in_f], F32, kind="ExternalInput")
    # interleaved weight shard: row 64*n + j (j<64) = full-weight row
    # 512*n + 64*c + j on core c
    w = nc.dram_tensor(
        "w_shard", [n_tiles * O_SLICE, in_f], F32, kind="ExternalInput"
    )
    b = nc.dram_tensor("bias", [out_f], F32, kind="ExternalInput")
    out = nc.dram_tensor("out", [t_shard, out_f], F32, kind="ExternalOutput")

    in_bs = [
        nc.dram_tensor(
            f"in_b{g}", [G, P, ko_tiles, O_SLICE], FP8, kind="Internal"
        )
        for g in range(n_tiles // G)
    ]
    out_bs = [
        nc.dram_tensor(
            f"out_b{g}",
            [N_CORES, G, P, ko_tiles, O_SLICE],
            FP8,
            kind="Internal",
            addr_space="Shared",
        )
        for g in range(n_tiles // G)
    ]

    with tile.TileContext(nc) as tc:
        with (
            tc.tile_pool(name="consts", bufs=8) as const_pool,
            tc.tile_pool(name="stage", bufs=stage_bufs) as stage_pool,
            tc.tile_pool(name="wslab", bufs=2) as wslab_pool,
            tc.tile_pool(name="wtb", bufs=2) as wtb_pool,
            tc.tile_pool(name="slice", bufs=2) as slice_pool,
            tc.tile_pool(name="xtb", bufs=2) as xtb_pool,
            tc.tile_pool(name="xt", bufs=1) as xt_pool,
            tc.tile_pool(name="wt", bufs=wt_bufs) as wt_pool,
            tc.tile_pool(name="wtc", bufs=1) as wtc_pool,
            tc.tile_pool(name="out_sb", bufs=6) as out_pool,
            tc.tile_pool(name="ps", bufs=8, space="PSUM") as psum_pool,
        ):
          wt_const = {}
          if skip_wprep:
            # two shared dummy tiles (timing-only variant; numerics wrong)
            for i in range(2):
                wt_n = wtc_pool.tile(
                    [P, N_CORES, ko_tiles, O_SLICE], FP8,
                    name=f"wtc_{i}", tag=f"wtc{i}",
                )
                nc.gpsimd.memset(wt_n, 1.0)
                wt_const[i] = wt_n
            wt_const = {n: wt_const[n % 2] for n in range(n_tiles)}
          for _rep in range(repeat):
            biases = []

            def emit_biases():
                for n in range(n_tiles):
                    bias_rep = const_pool.tile(
                        [P, N_TILE], F32, name=f"bias_rep_{n}", tag="bias"
                    )
                    nc.gpsimd.dma_start(
                        bias_rep,
                        b[None, ts(n, N_TILE)].broadcast_to([P, N_TILE]),
                    )
                    biases.append(bias_rep)

            def prep_slice(n):
                """Sign my 64 rows of block n, publish via AllGather."""
                slab = wslab_pool.tile(
                    [O_SLICE, in_f], BF16, name=f"wslab_{n}", tag="wslab"
                )
                nc.gpsimd.dma_start(slab, w[ts(n, O_SLICE), :])
                wtb = wtb_pool.tile(
                    [P, ko_tiles, O_SLICE], BF16, name=f"wtb_{n}", tag="wtb"
                )
                # NOTE: transposes must stay on nc.sync
                nc.sync.dma_start(wtb, slab, transpose=True)
                sl = slice_pool.tile(
                    [P, ko_tiles, O_SLICE], FP8, name=f"sl_{n}", tag="slice"
                )
                nc.scalar.activation(
                    sl, wtb, mybir.ActivationFunctionType.Sign, scale=1.0e30
                )
                g, s = divmod(n, G)
                nc.scalar.dma_start(in_bs[g][s, :, :, :], sl)
                if s == G - 1:
                    nc.gpsimd.collective_compute(
                        "AllGather",
                        mybir.AluOpType.bypass,
                        replica_groups=[list(range(N_CORES))],
                        ins=[in_bs[g][:, :, :, :].opt()],
                        outs=[out_bs[g][:, :, :, :, :].opt()],
                    )

            def load_wt(n):
                """One contiguous-desc DMA pulls the gathered block into SBUF
                as [128i, 8c, 32k, 64o]; matmuls consume it via a 4-dim AP
                whose (c,o) stream order equals the true output column order
                64c+o (interleaved shard)."""
                wt_n = wt_pool.tile(
                    [P, N_CORES, ko_tiles, O_SLICE], FP8, name=f"wt_{n}", tag="wt"
                )
                g, s = divmod(n, G)
                nc.scalar.dma_start(
                    wt_n,
                    out_bs[g][:, s, :, :, :].transpose((1, 0, 2, 3)),
                )
                return wt_n

            # x destination tiles (fp8 hi + partial residual lo)
            xt_hi = xt_pool.tile(
                [P, ko_tiles, t_shard], FP8, name="xt_hi", tag="xt_hi"
            )
            xt_lo = xt_pool.tile(
                [P, nk_res, t_shard], FP8, name="xt_lo", tag="xt_lo"
            )

            def emit_x(m):
                slab = stage_pool.tile([P, in_f], BF16, name="xslab", tag="stage")
                nc.gpsimd.dma_start(slab, x[ts(m, P), :])
                xtb = xtb_pool.tile(
                    [P, ko_tiles, P], BF16, name=f"xtb_{m}", tag="xtb"
                )
                nc.sync.dma_start(xtb, slab, transpose=True)
                nc.vector.tensor_copy(xt_hi[:, :, ts(m, P)], xtb)
                nc.vector.tensor_tensor(
                    xt_lo[:, :, ts(m, P)],
                    xtb[:, :nk_res, :],
                    xt_hi[:, :nk_res, ts(m, P)],
                    mybir.AluOpType.subtract,
                )

            if skip_wprep:
                def prep_slice(n):
                    return
                def load_wt(n):
                    return wt_const[n]
            # ---- head: weight slice 0+1 published first (the CC round-trip
            # gates the first matmuls), x pipeline interleaved
            head = max(4, G)
            prep_slice(0)
            for m in range(m_tiles // 2):
                emit_x(m)
            # biases after the critical head (slice 0 + first x half): they
            # are not consumed until the first PSUM drain ~60us in, and on
            # the Pool queue ahead of the slab loads they add head latency
            emit_biases()
            prep_slice(1)
            for m in range(m_tiles // 2, m_tiles):
                emit_x(m)
            for n in range(2, head):
                prep_slice(n)
            wts = {i: load_wt(i) for i in range(min(wt_depth, n_tiles))}

            # ---- main loop over output blocks
            for n in range(n_tiles):
                if n + head < n_tiles:
                    prep_slice(n + head)
                if not late_load and n + wt_depth < n_tiles:
                    wts[n + wt_depth] = load_wt(n + wt_depth)
                wt_n = wts.pop(n)

                half = psum_half or max(1, m_tiles // 2)
                for g0 in range(0, m_tiles, half):
                    ms = range(g0, min(g0 + half, m_tiles))
                    psums = {
                        m: psum_pool.tile(
                            [P, N_TILE], F32, name=f"ps_{n}_{m}", tag="ps"
                        )
                        for m in ms
                    }
                    n_hi = (ko_tiles // 2) if not skip_mm else 1
                    n_lo = (nk_res // 2) if not skip_mm else 0
                    for j in range(n_hi):
                        for m in ms:
                            nc.tensor.matmul(
                                psums[m],
                                xt_hi[:, 2 * j : 2 * j + 2, ts(m, P)],
                                wt_n[:, :, 2 * j : 2 * j + 2, :].transpose(
                                    (0, 2, 1, 3)
                                ),
                                start=(j == 0),
                                stop=(n_lo == 0 and j == n_hi - 1),
                                perf_mode=DR,
                            )
                    for j in range(n_lo):
                        for m in ms:
                            nc.tensor.matmul(
                                psums[m],
                                xt_lo[:, 2 * j : 2 * j + 2, ts(m, P)],
                                wt_n[:, :, 2 * j : 2 * j + 2, :].transpose(
                                    (0, 2, 1, 3)
                                ),
                                start=False,
                                stop=(j == n_lo - 1),
                                perf_mode=DR,
                            )
                    for m in ms:
                        out_sb = out_pool.tile(
                            [P, N_TILE], F32, name="out_sb", tag="out_sb"
                        )
                        nc.vector.tensor_tensor(
                            out_sb, psums[m], biases[n], mybir.AluOpType.add
                        )
                        (nc.sync if out_sync else nc.scalar).dma_start(
                            out[ts(m, P), ts(n, N_TILE)], out_sb
                        )
                if late_load and n + wt_depth < n_tiles:
                    wts[n + wt_depth] = load_wt(n + wt_depth)

    nc.compile()
    return nc


def make_in_maps(x, weight, bias):
    t_shard = x.shape[0] // N_CORES
    n_tiles = weight.shape[0] // N_TILE
    maps = []
    for c in range(N_CORES):
        idx = np.concatenate(
            [
                np.arange(O_SLICE) + N_TILE * n + O_SLICE * c
                for n in range(n_tiles)
            ]
        )
        maps.append(
            {
                "x": x[c * t_shard : (c + 1) * t_shard],
                "w_shard": np.ascontiguousarray(weight[idx]),
                "bias": bias,
            }
        )
    return maps


_NC_CACHE = {}


def _get_nc(shape_key):
    if shape_key not in _NC_CACHE:
        _NC_CACHE[shape_key] = build_nc(*shape_key)
    return _NC_CACHE[shape_key]


def _run(nc, x, weight, bias, trace):
    from concourse.bass_utils import run_bass_kernel_spmd

    res = run_bass_kernel_spmd(
        nc,
        make_in_maps(x, weight, bias),
        core_ids=list(range(N_CORES)),
        trace=trace,
    )
    return np.concatenate([r["out"] for r in res.results], axis=0), res


def _spot_check(out, x, weight, bias):
    """Verify 2 sampled output columns against numpy; guards against the
    rare transient where one core's execution returns zeros/garbage."""
    cols = [137, 3972]
    s = np.sign(weight[cols, :].astype(np.float32)).T  # [in_f, 2]
    ref = x.astype(np.float32) @ s + bias[cols][None, :]
    got = out[:, cols]
    denom = np.linalg.norm(ref)
    rel = np.linalg.norm(got - ref) / max(denom, 1e-30)
    return rel < 3e-2


def kernel(x, weight, bias, _trace=False):
    x = np.ascontiguousarray(np.asarray(x, dtype=np.float32))
    weight = np.ascontiguousarray(np.asarray(weight, dtype=np.float32))
    bias = np.ascontiguousarray(np.asarray(bias, dtype=np.float32))

    tokens = x.shape[0]
    t_shard = tokens // N_CORES
    nc = _get_nc((t_shard, x.shape[1], weight.shape[0]))

    out, res = _run(nc, x, weight, bias, _trace)
    if not _spot_check(out, x, weight, bias):
        # transient device-side failure - run once more
        out, res = _run(nc, x, weight, bias, _trace)
    if _trace:
        return out, res
    return out
